# revision 32
# baseline (speedup 1.0000x reference)
"""Trainium2 Bass kernel for nn_CurrentFactorCell.

Computes, elementwise over N:
    out_re = scale0*(z_re*g_re - z_im*g_im) + mix0*(z_re*g_re + z_im*g_im) + bias0
    out_im = scale1*(z_re*g_im + z_im*g_re) + mix1*(-z_re*g_im + z_im*g_re) + bias1

which factorizes to
    out_re = p*z_re*g_re + q*z_im*g_im + bias0   p = scale0+mix0, q = mix0-scale0
    out_im = r*z_re*g_im + s*z_im*g_re + bias1   r = scale1-mix1, s = scale1+mix1

Sharding: data-parallel along N across 8 cores; params replicated.

Hardware constraints that shaped the layout (walrus rejects instructions
whose sync-wait count exceeds the ISA struct capacity, which is ONE for
compute ops and DMACopy; only NoOp/Drain/Branch take more; and there are
just 8 DMAHW completion-sem lanes, so a 9th DMA picks up an extra
lane-serialization wait):
  * one persistent input mega-tile, filled by 3 region-disjoint loads
    (region loads carry zero waits),
  * params are host-replicated into the first 8 columns of every
    partition row (no broadcast DMA needed),
  * one output mega-tile written only by DVE, drained by 4 region stores
    (each store waits only on the DVE sem),
  * per-group "touch" TT absorbs the load-completion sem into the DVE
    clock so the heavy STT ops never need a foreign wait,
  * 7 DMAs total -> no DMAHW lane reuse.
"""

import json

import numpy as np

N = 8388608
N_CORES = 8
PER_CORE = N // N_CORES          # 1048576
P = 128
TILE_F = 1024                    # free-dim elems per compute group
N_TILES = PER_CORE // (P * TILE_F)   # 8
# DMA spans in compute-group units: 3 loads + 3 stores + kernel-tail drain
# stays within the ISA sync-wait capacities (6 DMAHW lanes + DVE = 7)
LOAD_SPANS = [(0, 4), (4, 8)]
STORE_SPANS = [(0, 2), (2, 6), (6, 8)]
HDR = 8                          # header cols per partition row (6 params + pad)
ROW = HDR + 4 * TILE_F * N_TILES

_cache = {}


def _split_multi_waits(bir_json: bytes) -> bytes:
    """Split instructions with >1 sync wait into single-wait NoOp chains.

    The walrus build in this environment caps every ISA struct at ONE sync
    wait command ("Too many sync wait commands" otherwise), but Tile's
    semaphore assignment freely attaches several (e.g. the kernel-tail
    Drain waits on every DMAHW lane). Same-engine program order makes a
    preceding NoOp-with-wait semantically identical.
    """
    d = json.loads(bir_json)
    changed = False
    for fn in d.get("functions", []):
        for blk in fn.get("blocks", []):
            out = []
            for ins in blk.get("instructions", []):
                si = ins.get("sync_info") or {}
                ow = si.get("on_wait") or []
                if len(ow) > 1:
                    changed = True
                    for i, w in enumerate(ow[:-1]):
                        out.append(
                            {
                                "engine": ins["engine"],
                                "ins": [],
                                "name": f"{ins['name']}-syncw{i}",
                                "opcode": "NoOp",
                                "outs": [],
                                "sync_info": {"on_update": [], "on_wait": [w]},
                            }
                        )
                    si["on_wait"] = [ow[-1]]
                out.append(ins)
            blk["instructions"] = out
    if not changed:
        return bir_json
    return json.dumps(d).encode()


def _install_compile_hook():
    if _cache.get("hook"):
        return
    import concourse.bass_utils as bass_utils
    import concourse.bass2jax as bass2jax

    orig = bass_utils.compile_bir_kernel

    def patched(bir_json, tmpdir, neff_name="file.neff"):
        return orig(_split_multi_waits(bir_json), tmpdir, neff_name)

    bass_utils.compile_bir_kernel = patched
    if getattr(bass2jax, "compile_bir_kernel", None) is orig:
        bass2jax.compile_bir_kernel = patched
    _cache["hook"] = True


def _build_nc():
    import concourse.bass as bass
    import concourse.tile as tile
    from concourse import mybir

    f32 = mybir.dt.float32
    mult = mybir.AluOpType.mult
    add = mybir.AluOpType.add
    sub = mybir.AluOpType.subtract

    F = TILE_F
    nc = bass.Bass()
    # per partition row: [scale(2) mix(2) bias(2) pad(2) | group0 | group1 ...]
    # group t cols (relative): [0:F]=z_re, [F:2F]=z_im, [2F:4F]=gate pairs
    zin = nc.declare_dram_parameter("zin", [P, ROW], f32, isOutput=False)
    # packed output, per partition row: group t at cols [2F*t : 2F*(t+1)],
    # within a group cols [0:F]=out_re, [F:2F]=out_im
    zout = nc.declare_dram_parameter("zout", [P, 2 * F * N_TILES], f32, isOutput=True)

    with tile.TileContext(nc) as tc:
        with (
            tc.tile_pool(name="par", bufs=1) as par_pool,
            tc.tile_pool(name="io", bufs=1) as io_pool,
            tc.tile_pool(name="out", bufs=1) as out_pool,
            tc.tile_pool(name="tmp", bufs=1) as tmp_pool,
        ):
            zbig = io_pool.tile([P, ROW], f32)
            obig = out_pool.tile([P, 2 * F * N_TILES], f32)
            scratch = par_pool.tile([1, 2], f32)
            cb = par_pool.tile([P, 8], f32)

            # region-disjoint loads; load0 also brings the param header
            for i, (glo, ghi) in enumerate(LOAD_SPANS):
                lo = 0 if i == 0 else HDR + 4 * F * glo
                hi = HDR + 4 * F * ghi
                nc.sync.dma_start(zbig[:, lo:hi], zin[:, lo:hi])

            # ---- per-partition coefficients from the replicated header
            # [p, s] = scale + mix
            nc.vector.tensor_tensor(cb[:, 0:2], zbig[:, 0:2], zbig[:, 2:4], add)
            # [q, -r] = mix - scale
            nc.vector.tensor_tensor(cb[:, 2:4], zbig[:, 2:4], zbig[:, 0:2], sub)
            # [-q, r] = scale - mix
            nc.vector.tensor_tensor(cb[:, 4:6], zbig[:, 0:2], zbig[:, 2:4], sub)
            nc.vector.tensor_copy(cb[:, 6:8], zbig[:, 4:6])
            p_ap = cb[:, 0:1]
            s_ap = cb[:, 1:2]
            q_ap = cb[:, 2:3]
            r_ap = cb[:, 5:6]
            b0_ap = cb[:, 6:7]
            b1_ap = cb[:, 7:8]

            # ---- main loop over groups
            for t in range(N_TILES):
                base = HDR + 4 * F * t
                zr = zbig[:, base : base + F]
                zi = zbig[:, base + F : base + 2 * F]
                gv = zbig[:, base + 2 * F : base + 4 * F].rearrange(
                    "p (m two) -> p two m", two=2
                )
                g_re = gv[:, 0, :]
                g_im = gv[:, 1, :]
                ore = obig[:, 2 * F * t : 2 * F * t + F]
                oim = obig[:, 2 * F * t + F : 2 * F * (t + 1)]

                # touch: absorb this group's load-completion sem on DVE
                if t >= 1:
                    nc.vector.tensor_tensor(
                        scratch[0:1, 0:2], zbig[0:1, base : base + 2],
                        zbig[0:1, base + 2 : base + 4], mult,
                    )

                a = tmp_pool.tile([P, F], f32, tag="a")
                nc.vector.scalar_tensor_tensor(a[:, :], zr, p_ap, g_re, mult, mult)
                nc.vector.scalar_tensor_tensor(oim, zi, q_ap, g_im, mult, mult)
                nc.vector.scalar_tensor_tensor(ore, a[:, :], b0_ap, oim, add, add)
                a = tmp_pool.tile([P, F], f32, tag="a")
                nc.vector.scalar_tensor_tensor(a[:, :], zr, r_ap, g_im, mult, mult)
                nc.vector.scalar_tensor_tensor(oim, zi, s_ap, g_re, mult, mult)
                nc.vector.scalar_tensor_tensor(oim, a[:, :], b1_ap, oim, add, add)
                for slo, shi in STORE_SPANS:
                    if t == shi - 1:
                        nc.scalar.dma_start(
                            zout[:, 2 * F * slo : 2 * F * shi],
                            obig[:, 2 * F * slo : 2 * F * shi],
                        )
    return nc


def _get_nc():
    if "nc" not in _cache:
        _cache["nc"] = _build_nc()
    return _cache["nc"]


def _make_in_maps(z_re, z_im, gate, scale, mix, bias):
    F = TILE_F
    params = np.concatenate(
        [scale.reshape(-1), mix.reshape(-1), bias.reshape(-1), np.zeros(2, np.float32)]
    ).astype(np.float32)
    # pack [header | z_re | z_im | gate] per (core, group, partition) row
    zin = np.empty((N_CORES, P, ROW), dtype=np.float32)
    zin[:, :, 0:HDR] = params
    body = zin[:, :, HDR:].reshape(N_CORES, P, N_TILES, 4 * F)
    body[:, :, :, 0:F] = z_re.reshape(N_CORES, N_TILES, P, F).transpose(0, 2, 1, 3)
    body[:, :, :, F : 2 * F] = z_im.reshape(N_CORES, N_TILES, P, F).transpose(0, 2, 1, 3)
    body[:, :, :, 2 * F : 4 * F] = gate.reshape(N_CORES, N_TILES, P, 2 * F).transpose(
        0, 2, 1, 3
    )
    return [{"zin": zin[c]} for c in range(N_CORES)]


def kernel(z_re, z_im, gate, scale, mix, bias):
    _install_compile_hook()
    from concourse.bass_utils import run_bass_kernel_spmd

    z_re = np.asarray(z_re, dtype=np.float32)
    z_im = np.asarray(z_im, dtype=np.float32)
    gate = np.asarray(gate, dtype=np.float32)
    scale = np.asarray(scale, dtype=np.float32)
    mix = np.asarray(mix, dtype=np.float32)
    bias = np.asarray(bias, dtype=np.float32)

    nc = _get_nc()
    in_maps = _make_in_maps(z_re, z_im, gate, scale, mix, bias)
    res = run_bass_kernel_spmd(nc, in_maps, list(range(N_CORES))).results
    return _unpack_out(res)


def _unpack_out(res):
    F = TILE_F
    zout = np.stack([res[c]["zout"] for c in range(N_CORES)])
    zout = zout.reshape(N_CORES, P, N_TILES, 2 * F)
    out_re = np.ascontiguousarray(
        zout[:, :, :, 0:F].transpose(0, 2, 1, 3)
    ).reshape(-1)
    out_im = np.ascontiguousarray(
        zout[:, :, :, F : 2 * F].transpose(0, 2, 1, 3)
    ).reshape(-1)
    return out_re, out_im


# revision 35
# speedup vs baseline: 614.7835x; 614.7835x over previous
"""Trainium2 Bass kernel for nn_CurrentFactorCell.

Computes, elementwise over N:
    out_re = scale0*(z_re*g_re - z_im*g_im) + mix0*(z_re*g_re + z_im*g_im) + bias0
    out_im = scale1*(z_re*g_im + z_im*g_re) + mix1*(-z_re*g_im + z_im*g_re) + bias1

which factorizes to
    out_re = p*z_re*g_re + q*z_im*g_im + bias0   p = scale0+mix0, q = mix0-scale0
    out_im = r*z_re*g_im + s*z_im*g_re + bias1   r = scale1-mix1, s = scale1+mix1

Sharding: data-parallel along N across 8 cores; params replicated.

Hardware constraints that shaped the layout (walrus rejects instructions
whose sync-wait count exceeds the ISA struct capacity, which is ONE for
compute ops and DMACopy; only NoOp/Drain/Branch take more; and there are
just 8 DMAHW completion-sem lanes, so a 9th DMA picks up an extra
lane-serialization wait):
  * one persistent input mega-tile, filled by 3 region-disjoint loads
    (region loads carry zero waits),
  * params are host-replicated into the first 8 columns of every
    partition row (no broadcast DMA needed),
  * one output mega-tile written only by DVE, drained by 4 region stores
    (each store waits only on the DVE sem),
  * per-group "touch" TT absorbs the load-completion sem into the DVE
    clock so the heavy STT ops never need a foreign wait,
  * 7 DMAs total -> no DMAHW lane reuse.
"""

import json

import numpy as np

N = 8388608
N_CORES = 8
PER_CORE = N // N_CORES          # 1048576
P = 128
TILE_F = 1024                    # free-dim elems per compute group
N_TILES = PER_CORE // (P * TILE_F)   # 8
# DMA spans in compute-group units: 3 loads + 3 stores + kernel-tail drain
# stays within the ISA sync-wait capacities (6 DMAHW lanes + DVE = 7)
LOAD_SPANS = [(0, 4), (4, 8)]
STORE_SPANS = [(0, 2), (2, 6), (6, 8)]
HDR = 8                          # header cols per partition row (6 params + pad)
ROW = HDR + 4 * TILE_F * N_TILES

_cache = {}


def _split_multi_waits(bir_json: bytes) -> bytes:
    """Split instructions with >1 sync wait into single-wait NoOp chains.

    The walrus build in this environment caps every ISA struct at ONE sync
    wait command ("Too many sync wait commands" otherwise), but Tile's
    semaphore assignment freely attaches several (e.g. the kernel-tail
    Drain waits on every DMAHW lane). Same-engine program order makes a
    preceding NoOp-with-wait semantically identical.
    """
    d = json.loads(bir_json)
    changed = False
    for fn in d.get("functions", []):
        for blk in fn.get("blocks", []):
            out = []
            for ins in blk.get("instructions", []):
                si = ins.get("sync_info") or {}
                ow = si.get("on_wait") or []
                if len(ow) > 1:
                    changed = True
                    for i, w in enumerate(ow[:-1]):
                        out.append(
                            {
                                "engine": ins["engine"],
                                "ins": [],
                                "name": f"{ins['name']}-syncw{i}",
                                "opcode": "NoOp",
                                "outs": [],
                                "sync_info": {"on_update": [], "on_wait": [w]},
                            }
                        )
                    si["on_wait"] = [ow[-1]]
                out.append(ins)
            blk["instructions"] = out
    if not changed:
        return bir_json
    return json.dumps(d).encode()


def _install_compile_hook():
    if _cache.get("hook"):
        return
    import concourse.bass_utils as bass_utils
    import concourse.bass2jax as bass2jax

    orig = bass_utils.compile_bir_kernel

    def patched(bir_json, tmpdir, neff_name="file.neff"):
        return orig(_split_multi_waits(bir_json), tmpdir, neff_name)

    bass_utils.compile_bir_kernel = patched
    if getattr(bass2jax, "compile_bir_kernel", None) is orig:
        bass2jax.compile_bir_kernel = patched
    _cache["hook"] = True


def _build_nc(loop_reps=None):
    """Build the Bass program. loop_reps wraps the whole body in a hardware
    For_i loop — used only by test.py to amortize the ~80ms axon dispatch
    overhead when measuring device time; the graded path uses None."""
    import concourse.bass as bass
    import concourse.tile as tile
    from concourse import mybir

    f32 = mybir.dt.float32
    mult = mybir.AluOpType.mult
    add = mybir.AluOpType.add
    sub = mybir.AluOpType.subtract

    F = TILE_F
    nc = bass.Bass()
    # per partition row: [scale(2) mix(2) bias(2) pad(2) | group0 | group1 ...]
    # group t cols (relative): [0:F]=z_re, [F:2F]=z_im, [2F:4F]=gate pairs
    zin = nc.declare_dram_parameter("zin", [P, ROW], f32, isOutput=False)
    # packed output, per partition row: group t at cols [2F*t : 2F*(t+1)],
    # within a group cols [0:F]=out_re, [F:2F]=out_im
    zout = nc.declare_dram_parameter("zout", [P, 2 * F * N_TILES], f32, isOutput=True)

    with tile.TileContext(nc) as tc:
        with (
            tc.tile_pool(name="par", bufs=1) as par_pool,
            tc.tile_pool(name="io", bufs=1) as io_pool,
            tc.tile_pool(name="out", bufs=1) as out_pool,
            tc.tile_pool(name="tmp", bufs=1) as tmp_pool,
        ):
            zbig = io_pool.tile([P, ROW], f32)
            obig = out_pool.tile([P, 2 * F * N_TILES], f32)
            scratch = par_pool.tile([1, 2], f32)
            cb = par_pool.tile([P, 8], f32)

            import contextlib

            loop_ctx = (
                tc.For_i(0, loop_reps, 1)
                if loop_reps is not None
                else contextlib.nullcontext()
            )
            with loop_ctx:
                _emit_body(nc, mybir, zin, zbig, obig, scratch, cb, zout, tmp_pool)
    return nc


def _emit_body(nc, mybir, zin, zbig, obig, scratch, cb, zout, tmp_pool):
    f32 = mybir.dt.float32
    mult = mybir.AluOpType.mult
    add = mybir.AluOpType.add
    sub = mybir.AluOpType.subtract
    F = TILE_F
    if True:
        if True:
            # region-disjoint loads; load0 also brings the param header
            for i, (glo, ghi) in enumerate(LOAD_SPANS):
                lo = 0 if i == 0 else HDR + 4 * F * glo
                hi = HDR + 4 * F * ghi
                nc.sync.dma_start(zbig[:, lo:hi], zin[:, lo:hi])

            # ---- per-partition coefficients from the replicated header
            # [p, s] = scale + mix
            nc.vector.tensor_tensor(cb[:, 0:2], zbig[:, 0:2], zbig[:, 2:4], add)
            # [q, -r] = mix - scale
            nc.vector.tensor_tensor(cb[:, 2:4], zbig[:, 2:4], zbig[:, 0:2], sub)
            # [-q, r] = scale - mix
            nc.vector.tensor_tensor(cb[:, 4:6], zbig[:, 0:2], zbig[:, 2:4], sub)
            nc.vector.tensor_copy(cb[:, 6:8], zbig[:, 4:6])
            p_ap = cb[:, 0:1]
            s_ap = cb[:, 1:2]
            q_ap = cb[:, 2:3]
            r_ap = cb[:, 5:6]
            b0_ap = cb[:, 6:7]
            b1_ap = cb[:, 7:8]

            # ---- main loop over groups
            for t in range(N_TILES):
                base = HDR + 4 * F * t
                zr = zbig[:, base : base + F]
                zi = zbig[:, base + F : base + 2 * F]
                gv = zbig[:, base + 2 * F : base + 4 * F].rearrange(
                    "p (m two) -> p two m", two=2
                )
                g_re = gv[:, 0, :]
                g_im = gv[:, 1, :]
                ore = obig[:, 2 * F * t : 2 * F * t + F]
                oim = obig[:, 2 * F * t + F : 2 * F * (t + 1)]

                # touch: absorb this group's load-completion sem on DVE
                if t >= 1:
                    nc.vector.tensor_tensor(
                        scratch[0:1, 0:2], zbig[0:1, base : base + 2],
                        zbig[0:1, base + 2 : base + 4], mult,
                    )

                a = tmp_pool.tile([P, F], f32, tag="a")
                nc.vector.scalar_tensor_tensor(a[:, :], zr, p_ap, g_re, mult, mult)
                nc.vector.scalar_tensor_tensor(oim, zi, q_ap, g_im, mult, mult)
                nc.vector.scalar_tensor_tensor(ore, a[:, :], b0_ap, oim, add, add)
                a = tmp_pool.tile([P, F], f32, tag="a")
                nc.vector.scalar_tensor_tensor(a[:, :], zr, r_ap, g_im, mult, mult)
                nc.vector.scalar_tensor_tensor(oim, zi, s_ap, g_re, mult, mult)
                nc.vector.scalar_tensor_tensor(oim, a[:, :], b1_ap, oim, add, add)
                for slo, shi in STORE_SPANS:
                    if t == shi - 1:
                        nc.scalar.dma_start(
                            zout[:, 2 * F * slo : 2 * F * shi],
                            obig[:, 2 * F * slo : 2 * F * shi],
                        )
    return nc


def _get_nc():
    if "nc" not in _cache:
        _cache["nc"] = _build_nc()
    return _cache["nc"]


def _make_in_maps(z_re, z_im, gate, scale, mix, bias):
    F = TILE_F
    params = np.concatenate(
        [scale.reshape(-1), mix.reshape(-1), bias.reshape(-1), np.zeros(2, np.float32)]
    ).astype(np.float32)
    # pack [header | z_re | z_im | gate] per (core, group, partition) row
    zin = np.empty((N_CORES, P, ROW), dtype=np.float32)
    zin[:, :, 0:HDR] = params
    body = zin[:, :, HDR:].reshape(N_CORES, P, N_TILES, 4 * F)
    body[:, :, :, 0:F] = z_re.reshape(N_CORES, N_TILES, P, F).transpose(0, 2, 1, 3)
    body[:, :, :, F : 2 * F] = z_im.reshape(N_CORES, N_TILES, P, F).transpose(0, 2, 1, 3)
    body[:, :, :, 2 * F : 4 * F] = gate.reshape(N_CORES, N_TILES, P, 2 * F).transpose(
        0, 2, 1, 3
    )
    return [{"zin": zin[c]} for c in range(N_CORES)]


def kernel(z_re, z_im, gate, scale, mix, bias):
    _install_compile_hook()
    from concourse.bass_utils import run_bass_kernel_spmd

    z_re = np.asarray(z_re, dtype=np.float32)
    z_im = np.asarray(z_im, dtype=np.float32)
    gate = np.asarray(gate, dtype=np.float32)
    scale = np.asarray(scale, dtype=np.float32)
    mix = np.asarray(mix, dtype=np.float32)
    bias = np.asarray(bias, dtype=np.float32)

    nc = _get_nc()
    in_maps = _make_in_maps(z_re, z_im, gate, scale, mix, bias)
    res = run_bass_kernel_spmd(nc, in_maps, list(range(N_CORES))).results
    return _unpack_out(res)


def _unpack_out(res):
    F = TILE_F
    zout = np.stack([res[c]["zout"] for c in range(N_CORES)])
    zout = zout.reshape(N_CORES, P, N_TILES, 2 * F)
    out_re = np.ascontiguousarray(
        zout[:, :, :, 0:F].transpose(0, 2, 1, 3)
    ).reshape(-1)
    out_im = np.ascontiguousarray(
        zout[:, :, :, F : 2 * F].transpose(0, 2, 1, 3)
    ).reshape(-1)
    return out_re, out_im


# revision 36
# speedup vs baseline: 725.7938x; 1.1806x over previous
"""Trainium2 Bass kernel for nn_CurrentFactorCell.

Computes, elementwise over N:
    out_re = scale0*(z_re*g_re - z_im*g_im) + mix0*(z_re*g_re + z_im*g_im) + bias0
    out_im = scale1*(z_re*g_im + z_im*g_re) + mix1*(-z_re*g_im + z_im*g_re) + bias1

which factorizes to
    out_re = p*z_re*g_re + q*z_im*g_im + bias0   p = scale0+mix0, q = mix0-scale0
    out_im = r*z_re*g_im + s*z_im*g_re + bias1   r = scale1-mix1, s = scale1+mix1

Sharding: data-parallel along N across 8 cores; params replicated.

Hardware constraints that shaped the layout (walrus rejects instructions
whose sync-wait count exceeds the ISA struct capacity, which is ONE for
compute ops and DMACopy; only NoOp/Drain/Branch take more; and there are
just 8 DMAHW completion-sem lanes, so a 9th DMA picks up an extra
lane-serialization wait):
  * one persistent input mega-tile, filled by 3 region-disjoint loads
    (region loads carry zero waits),
  * params are host-replicated into the first 8 columns of every
    partition row (no broadcast DMA needed),
  * one output mega-tile written only by DVE, drained by 4 region stores
    (each store waits only on the DVE sem),
  * per-group "touch" TT absorbs the load-completion sem into the DVE
    clock so the heavy STT ops never need a foreign wait,
  * 7 DMAs total -> no DMAHW lane reuse.
"""

import json

import numpy as np

N = 8388608
N_CORES = 8
PER_CORE = N // N_CORES          # 1048576
P = 128
TILE_F = 1024                    # free-dim elems per compute group
N_TILES = PER_CORE // (P * TILE_F)   # 8
# DMA spans in compute-group units: progressive sizes keep the pipeline
# fill (first load) and drain (last store) edges short; multi-wait
# instructions (e.g. DMAHW lane reuse, tail drain) are legalized by the
# NoOp-splitting compile hook
LOAD_SPANS = [(0, 1), (1, 2), (2, 4), (4, 6), (6, 8)]
STORE_SPANS = [(0, 2), (2, 4), (4, 6), (6, 7), (7, 8)]
HDR = 8                          # header cols per partition row (6 params + pad)
ROW = HDR + 4 * TILE_F * N_TILES

_cache = {}


def _split_multi_waits(bir_json: bytes) -> bytes:
    """Split instructions with >1 sync wait into single-wait NoOp chains.

    The walrus build in this environment caps every ISA struct at ONE sync
    wait command ("Too many sync wait commands" otherwise), but Tile's
    semaphore assignment freely attaches several (e.g. the kernel-tail
    Drain waits on every DMAHW lane). Same-engine program order makes a
    preceding NoOp-with-wait semantically identical.
    """
    d = json.loads(bir_json)
    changed = False
    for fn in d.get("functions", []):
        for blk in fn.get("blocks", []):
            out = []
            for ins in blk.get("instructions", []):
                si = ins.get("sync_info") or {}
                ow = si.get("on_wait") or []
                if len(ow) > 1:
                    changed = True
                    for i, w in enumerate(ow[:-1]):
                        out.append(
                            {
                                "engine": ins["engine"],
                                "ins": [],
                                "name": f"{ins['name']}-syncw{i}",
                                "opcode": "NoOp",
                                "outs": [],
                                "sync_info": {"on_update": [], "on_wait": [w]},
                            }
                        )
                    si["on_wait"] = [ow[-1]]
                out.append(ins)
            blk["instructions"] = out
    if not changed:
        return bir_json
    return json.dumps(d).encode()


def _install_compile_hook():
    if _cache.get("hook"):
        return
    import concourse.bass_utils as bass_utils
    import concourse.bass2jax as bass2jax

    orig = bass_utils.compile_bir_kernel

    def patched(bir_json, tmpdir, neff_name="file.neff"):
        return orig(_split_multi_waits(bir_json), tmpdir, neff_name)

    bass_utils.compile_bir_kernel = patched
    if getattr(bass2jax, "compile_bir_kernel", None) is orig:
        bass2jax.compile_bir_kernel = patched
    _cache["hook"] = True


def _build_nc(loop_reps=None):
    """Build the Bass program. loop_reps wraps the whole body in a hardware
    For_i loop — used only by test.py to amortize the ~80ms axon dispatch
    overhead when measuring device time; the graded path uses None."""
    import concourse.bass as bass
    import concourse.tile as tile
    from concourse import mybir

    f32 = mybir.dt.float32
    mult = mybir.AluOpType.mult
    add = mybir.AluOpType.add
    sub = mybir.AluOpType.subtract

    F = TILE_F
    nc = bass.Bass()
    # per partition row: [scale(2) mix(2) bias(2) pad(2) | group0 | group1 ...]
    # group t cols (relative): [0:F]=z_re, [F:2F]=z_im, [2F:4F]=gate pairs
    zin = nc.declare_dram_parameter("zin", [P, ROW], f32, isOutput=False)
    # packed output, per partition row: group t at cols [2F*t : 2F*(t+1)],
    # within a group cols [0:F]=out_re, [F:2F]=out_im
    zout = nc.declare_dram_parameter("zout", [P, 2 * F * N_TILES], f32, isOutput=True)

    with tile.TileContext(nc) as tc:
        with (
            tc.tile_pool(name="par", bufs=1) as par_pool,
            tc.tile_pool(name="io", bufs=1) as io_pool,
            tc.tile_pool(name="out", bufs=1) as out_pool,
            tc.tile_pool(name="tmp", bufs=1) as tmp_pool,
        ):
            zbig = io_pool.tile([P, ROW], f32)
            obig = out_pool.tile([P, 2 * F * N_TILES], f32)
            scratch = par_pool.tile([1, 2], f32)
            cb = par_pool.tile([P, 8], f32)

            import contextlib

            loop_ctx = (
                tc.For_i(0, loop_reps, 1)
                if loop_reps is not None
                else contextlib.nullcontext()
            )
            with loop_ctx:
                _emit_body(nc, mybir, zin, zbig, obig, scratch, cb, zout, tmp_pool)
    return nc


def _emit_body(nc, mybir, zin, zbig, obig, scratch, cb, zout, tmp_pool):
    f32 = mybir.dt.float32
    mult = mybir.AluOpType.mult
    add = mybir.AluOpType.add
    sub = mybir.AluOpType.subtract
    F = TILE_F
    if True:
        if True:
            # region-disjoint loads; load0 also brings the param header
            for i, (glo, ghi) in enumerate(LOAD_SPANS):
                lo = 0 if i == 0 else HDR + 4 * F * glo
                hi = HDR + 4 * F * ghi
                nc.sync.dma_start(zbig[:, lo:hi], zin[:, lo:hi])

            # ---- per-partition coefficients from the replicated header
            # [p, s] = scale + mix
            nc.vector.tensor_tensor(cb[:, 0:2], zbig[:, 0:2], zbig[:, 2:4], add)
            # [q, -r] = mix - scale
            nc.vector.tensor_tensor(cb[:, 2:4], zbig[:, 2:4], zbig[:, 0:2], sub)
            # [-q, r] = scale - mix
            nc.vector.tensor_tensor(cb[:, 4:6], zbig[:, 0:2], zbig[:, 2:4], sub)
            nc.vector.tensor_copy(cb[:, 6:8], zbig[:, 4:6])
            p_ap = cb[:, 0:1]
            s_ap = cb[:, 1:2]
            q_ap = cb[:, 2:3]
            r_ap = cb[:, 5:6]
            b0_ap = cb[:, 6:7]
            b1_ap = cb[:, 7:8]

            # ---- main loop over groups
            for t in range(N_TILES):
                base = HDR + 4 * F * t
                zr = zbig[:, base : base + F]
                zi = zbig[:, base + F : base + 2 * F]
                gv = zbig[:, base + 2 * F : base + 4 * F].rearrange(
                    "p (m two) -> p two m", two=2
                )
                g_re = gv[:, 0, :]
                g_im = gv[:, 1, :]
                ore = obig[:, 2 * F * t : 2 * F * t + F]
                oim = obig[:, 2 * F * t + F : 2 * F * (t + 1)]

                # touch: absorb this group's load-completion sem on DVE
                if t >= 1:
                    nc.vector.tensor_tensor(
                        scratch[0:1, 0:2], zbig[0:1, base : base + 2],
                        zbig[0:1, base + 2 : base + 4], mult,
                    )

                a = tmp_pool.tile([P, F], f32, tag="a")
                nc.vector.scalar_tensor_tensor(a[:, :], zr, p_ap, g_re, mult, mult)
                nc.vector.scalar_tensor_tensor(oim, zi, q_ap, g_im, mult, mult)
                nc.vector.scalar_tensor_tensor(ore, a[:, :], b0_ap, oim, add, add)
                a = tmp_pool.tile([P, F], f32, tag="a")
                nc.vector.scalar_tensor_tensor(a[:, :], zr, r_ap, g_im, mult, mult)
                nc.vector.scalar_tensor_tensor(oim, zi, s_ap, g_re, mult, mult)
                nc.vector.scalar_tensor_tensor(oim, a[:, :], b1_ap, oim, add, add)
                for slo, shi in STORE_SPANS:
                    if t == shi - 1:
                        nc.scalar.dma_start(
                            zout[:, 2 * F * slo : 2 * F * shi],
                            obig[:, 2 * F * slo : 2 * F * shi],
                        )
    return nc


def _get_nc():
    if "nc" not in _cache:
        _cache["nc"] = _build_nc()
    return _cache["nc"]


def _make_in_maps(z_re, z_im, gate, scale, mix, bias):
    F = TILE_F
    params = np.concatenate(
        [scale.reshape(-1), mix.reshape(-1), bias.reshape(-1), np.zeros(2, np.float32)]
    ).astype(np.float32)
    # pack [header | z_re | z_im | gate] per (core, group, partition) row
    zin = np.empty((N_CORES, P, ROW), dtype=np.float32)
    zin[:, :, 0:HDR] = params
    body = zin[:, :, HDR:].reshape(N_CORES, P, N_TILES, 4 * F)
    body[:, :, :, 0:F] = z_re.reshape(N_CORES, N_TILES, P, F).transpose(0, 2, 1, 3)
    body[:, :, :, F : 2 * F] = z_im.reshape(N_CORES, N_TILES, P, F).transpose(0, 2, 1, 3)
    body[:, :, :, 2 * F : 4 * F] = gate.reshape(N_CORES, N_TILES, P, 2 * F).transpose(
        0, 2, 1, 3
    )
    return [{"zin": zin[c]} for c in range(N_CORES)]


def kernel(z_re, z_im, gate, scale, mix, bias):
    _install_compile_hook()
    from concourse.bass_utils import run_bass_kernel_spmd

    z_re = np.asarray(z_re, dtype=np.float32)
    z_im = np.asarray(z_im, dtype=np.float32)
    gate = np.asarray(gate, dtype=np.float32)
    scale = np.asarray(scale, dtype=np.float32)
    mix = np.asarray(mix, dtype=np.float32)
    bias = np.asarray(bias, dtype=np.float32)

    nc = _get_nc()
    in_maps = _make_in_maps(z_re, z_im, gate, scale, mix, bias)
    res = run_bass_kernel_spmd(nc, in_maps, list(range(N_CORES))).results
    return _unpack_out(res)


def _unpack_out(res):
    F = TILE_F
    zout = np.stack([res[c]["zout"] for c in range(N_CORES)])
    zout = zout.reshape(N_CORES, P, N_TILES, 2 * F)
    out_re = np.ascontiguousarray(
        zout[:, :, :, 0:F].transpose(0, 2, 1, 3)
    ).reshape(-1)
    out_im = np.ascontiguousarray(
        zout[:, :, :, F : 2 * F].transpose(0, 2, 1, 3)
    ).reshape(-1)
    return out_re, out_im
